# revision 1
# baseline (speedup 1.0000x reference)
"""Trainium2 Bass kernel for the neural 2D min-sum LDPC decoder problem.

Strategy (v2)
-------------
Data-parallel over the batch: B=512 codewords, 64 per NeuronCore (8 cores).
Per core, per-edge state lives in SBUF with the graph on the partition axis
and the 64-batch on the free axis (256B rows).

The Tanner graph (edge_v/edge_c) is 6-regular on checks, 3-regular on
variables, built from 3 "layers": sorting each check's edges by edge id
puts exactly one edge of every variable in slots {0,1}, {2,3}, {4,5}.
Variables are relabeled by their slot-{0,1} position, which makes the
layer-0 part of both crossings contiguous.

Per iteration:
  check phase   x_j = u_j - alpha_{t-1}*c2v_j (fused), then leave-one-out
                min + sign-product min-sum -> c2v (6 slot arrays
                [128,32,64]); slots 2..5 DMA'd contiguously to DRAM.
  crossing 1    4 dma_gathers (4096x256B) fetch, for every variable, the
                c2v of its layer-1 and layer-2 edges;
                u_var = llr + alpha_t*((c2v_l0 + g_mid) + g_hi).
  crossing 2    u_var written contiguously to DRAM; 4 dma_gathers
                redistribute it to slots 2..5 position order (layer 0 is
                contiguous by construction).
All gathers: 256B rows, 4 SWDGE queues round-robin, single_packet=False,
split in halves so the next phase starts on the first half early.
alpha/beta are baked as immediates (compiled after inputs are known).
"""

import sys

for _p in ("/opt/trn_rl_repo",):
    if _p not in sys.path:
        sys.path.insert(0, _p)

import numpy as np

import concourse.bass as bass
import concourse.bacc as bacc
import concourse.mybir as mybir
import concourse.tile as tile
from concourse.bass_utils import run_bass_kernel_spmd

N = 8192          # variable nodes
M = 4096          # check nodes
DC = 6            # check degree (slots)
DV = 3            # variable degree
E = N * DV
B = 512
T = 10
NCORES = 8
BL = B // NCORES  # 64
PB = 128
GB_ = M // PB     # 32 blocks per slot array
CHUNK_BLKS = 4
NCHUNK = GB_ // CHUNK_BLKS

F32 = mybir.dt.float32
I32 = mybir.dt.int32
I16 = mybir.dt.int16
ALU = mybir.AluOpType
ACTF = mybir.ActivationFunctionType


def _derive_graph(edge_v: np.ndarray, edge_c: np.ndarray):
    """Host-side index derivation (layered 6-regular/3-regular graph)."""
    edge_v = np.asarray(edge_v, dtype=np.int64)
    edge_c = np.asarray(edge_c, dtype=np.int64)
    assert edge_v.shape == (E,) and edge_c.shape == (E,)

    order = np.argsort(edge_c, kind="stable")
    assert (edge_c[order] == np.repeat(np.arange(M), DC)).all(), (
        "graph is not 6-regular on checks"
    )
    slot_edge = order.reshape(M, DC).T.copy()  # [DC, M] edge id at (slot j, check c)

    # per-edge position
    j_of_e = np.empty(E, dtype=np.int64)
    c_of_e = np.empty(E, dtype=np.int64)
    for j in range(DC):
        j_of_e[slot_edge[j]] = j
        c_of_e[slot_edge[j]] = np.arange(M)

    # each variable must have exactly one edge in slots {0,1}, {2,3}, {4,5}
    layer_of_e = j_of_e // 2
    ve = np.full((N, 3), -1, dtype=np.int64)
    for lay in range(3):
        sel = np.where(layer_of_e == lay)[0]
        vs = edge_v[sel]
        assert len(np.unique(vs)) == N, f"layer {lay} is not a permutation"
        ve[vs, lay] = sel
    assert (ve >= 0).all()

    # storage row helpers (p-major: row = (c%128)*32 + c//128)
    rowmaj = (c_of_e % PB) * GB_ + (c_of_e // PB)
    # c2v DRAM buffer holds slots 2..5 only
    cdrow = (j_of_e - 2) * M + rowmaj          # valid for slots 2..5
    # u/llr DRAM row of a variable = its slot-{0,1} position
    fr_of_e = j_of_e * M + rowmaj              # valid for slots 0..1
    fr_of_v = fr_of_e[ve[:, 0]]                # [N]

    # u-build gathers (dst = parity pi, list pos = check c): variable at
    # (j=pi, c) -> cdram rows of its layer-1 / layer-2 edges
    ix1 = np.empty((2, M), dtype=np.int16)
    ix2 = np.empty((2, M), dtype=np.int16)
    # crossing-2 gathers (dst slot j=2..5, list pos = c): udram row of v(j,c)
    ixu = np.empty((4, M), dtype=np.int16)
    for pi in range(2):
        e = slot_edge[pi]                      # layer-0 edge at (pi, c)
        v = edge_v[e]
        ix1[pi] = cdrow[ve[v, 1]]
        ix2[pi] = cdrow[ve[v, 2]]
    for j in range(2, DC):
        v = edge_v[slot_edge[j]]
        ixu[j - 2] = fr_of_v[v]

    # host llr/output mapping: variable id at each u/llr DRAM row
    vid_of_fr = np.empty(N, dtype=np.int64)
    vid_of_fr[fr_of_v] = np.arange(N)
    return ix1, ix2, ixu, vid_of_fr


def _wrap_idx(idx_m: np.ndarray) -> np.ndarray:
    """dma_gather index layout: list position k at [k%16, k//16],
    replicated across the 8 groups of 16 partitions."""
    w = idx_m.reshape(M // 16, 16).T
    return np.tile(w, (PB // 16, 1)).copy()


def _build_program(alpha: np.ndarray, beta: np.ndarray) -> bacc.Bacc:
    nc = bacc.Bacc(num_swdge_queues=4)

    llr_t = nc.dram_tensor("llr_t", [N, BL], F32, kind="ExternalInput").ap()
    ix1_d = nc.dram_tensor("ix1", [2, PB, M // 16], I16, kind="ExternalInput").ap()
    ix2_d = nc.dram_tensor("ix2", [2, PB, M // 16], I16, kind="ExternalInput").ap()
    ixu_d = nc.dram_tensor("ixu", [4, PB, M // 16], I16, kind="ExternalInput").ap()
    post_d = nc.dram_tensor("post", [2, PB, GB_, BL], F32, kind="ExternalOutput").ap()
    bits_d = nc.dram_tensor("bits", [2, PB, GB_, BL], I32, kind="ExternalOutput").ap()
    # c2v slots 2..5, ping-pong; u_var, ping-pong
    cdrs = [
        nc.dram_tensor("cda", [4 * M, BL], F32).ap(),
        nc.dram_tensor("cdb", [4 * M, BL], F32).ap(),
    ]
    udrs = [
        nc.dram_tensor("uda", [N, BL], F32).ap(),
        nc.dram_tensor("udb", [N, BL], F32).ap(),
    ]
    cdrv = [c.rearrange("(j p g) e -> j p g e", j=4, p=PB) for c in cdrs]
    udrv = [u.rearrange("(pi p g) e -> p pi g e", pi=2, p=PB) for u in udrs]
    bitv = bits_d.rearrange("pi p g e -> p pi g e")

    QN = [0]

    def qn():
        # one queue per DMA-sem lane-pair: Tile locks each SWDGE sem lane to
        # a single queue, and lanes are assigned round-robin per gather.
        q = (QN[0] % 8) // 2
        QN[0] += 1
        return q

    with tile.TileContext(nc) as tc:
        with (
            tc.tile_pool(name="persist", bufs=1) as pp,
            tc.tile_pool(name="gbp", bufs=2) as gbp,
            tc.tile_pool(name="bits", bufs=1) as bip,
            tc.tile_pool(name="tmp", bufs=1) as tp,
            tc.tile_pool(name="ps", bufs=1, space="PSUM") as psp,
        ):
            ix1 = [pp.tile([PB, M // 16], I16, tag=f"ix1{i}", name=f"ix1{i}") for i in range(2)]
            ix2 = [pp.tile([PB, M // 16], I16, tag=f"ix2{i}", name=f"ix2{i}") for i in range(2)]
            ixu = [pp.tile([PB, M // 16], I16, tag=f"ixu{i}", name=f"ixu{i}") for i in range(4)]
            for i in range(2):
                nc.sync.dma_start(ix1[i][:], ix1_d[i])
                nc.sync.dma_start(ix2[i][:], ix2_d[i])
            for i in range(4):
                nc.sync.dma_start(ixu[i][:], ixu_d[i])

            # llr in variable(-row) order, parity-split: [128, 2, 32, 64]
            LV = pp.tile([PB, 2, GB_, BL], F32, tag="lv", name="lv")
            nc.sync.dma_start(
                LV[:], llr_t.rearrange("(pi p g) e -> p pi g e", pi=2, p=PB)
            )
            # u in position order: slots 0/1 = u_var parities, 2..5 gathered
            U = pp.tile([PB, DC, GB_, BL], F32, tag="u", name="u")
            # c2v
            C = pp.tile([PB, DC, GB_, BL], F32, tag="c", name="c")

            # t=0: u slots 0,1 = llr (variable order); 2..5 gathered from llr_t
            nc.scalar.activation(U[:, 0:2, :, :], LV[:], ACTF.Copy)
            for h in range(2):
                for i in range(4):
                    nc.gpsimd.dma_gather(
                        U[:, 2 + i, h * 16 : (h + 1) * 16, :],
                        llr_t,
                        ixu[i][:, h * 128 : (h + 1) * 128],
                        M // 2, M // 2, BL,
                        single_packet=False, queue_num=qn(),
                    )

            def check_chunk(t, ck, beta_t, alpha_p):
                """min-sum check update for chunk ck (CHUNK_BLKS blocks),
                slot-fused instructions via strided/pair-swapped AP views."""
                b0 = ck * CHUNK_BLKS
                S1 = CHUNK_BLKS * BL
                cs = C[:, :, b0 : b0 + CHUNK_BLKS, :]
                us = U[:, :, b0 : b0 + CHUNK_BLKS, :]
                if t > 0:
                    xt = psp.tile([PB, DC, CHUNK_BLKS, BL], F32, tag="x", name="xt")
                    nc.vector.scalar_tensor_tensor(
                        xt[:], cs, -alpha_p, us, ALU.mult, ALU.add
                    )
                    xs = xt[:]
                else:
                    xs = us
                mg = tp.tile([PB, DC, CHUNK_BLKS, BL], F32, tag="m", name="mg")
                sg = tp.tile([PB, DC, CHUNK_BLKS, BL], F32, tag="s", name="sg")
                nc.scalar.activation(mg[:], xs, ACTF.Abs)
                nc.scalar.activation(sg[:], xs, ACTF.Sign)
                pp3 = tp.tile([PB, 3, CHUNK_BLKS, BL], F32, tag="p3", name="pp3")
                qq3 = tp.tile([PB, 3, CHUNK_BLKS, BL], F32, tag="q3", name="qq3")
                sp3 = tp.tile([PB, 3, CHUNK_BLKS, BL], F32, tag="sp3", name="sp3")
                bsp = tp.tile([PB, CHUNK_BLKS, BL], F32, tag="bsp", name="bsp")
                ex = psp.tile([PB, DC, CHUNK_BLKS, BL], F32, tag="x", name="ex")
                # pair mins / pair sign-products (even x odd slots, strided)
                nc.vector.tensor_tensor(pp3[:], mg[:, 0::2], mg[:, 1::2], ALU.min)
                nc.vector.tensor_tensor(sp3[:], sg[:, 0::2], sg[:, 1::2], ALU.mult)
                # leave-one-pair-out mins
                nc.vector.tensor_tensor(qq3[:, 0], pp3[:, 1], pp3[:, 2], ALU.min)
                nc.vector.tensor_tensor(qq3[:, 1], pp3[:, 0], pp3[:, 2], ALU.min)
                nc.vector.tensor_tensor(qq3[:, 2], pp3[:, 0], pp3[:, 1], ALU.min)
                # leave-one-out min: E[j] = min(M[partner(j)], Q[j//2])
                mv = mg[:]
                msw = bass.AP(
                    mv.tensor, mv.offset + S1,
                    [mv.ap[0], [2 * S1, 3], [-S1, 2], [1, S1]],
                )
                qb = (qq3[:].rearrange("p a b e -> p a (b e)")[:, :, None, :]
                      .to_broadcast([PB, 3, 2, S1]))
                nc.vector.tensor_tensor(
                    ex[:].rearrange("p (a b) c e -> p a b (c e)", a=3), msw, qb, ALU.min
                )
                # total sign product * beta
                nc.vector.tensor_tensor(bsp[:], sp3[:, 0], sp3[:, 1], ALU.mult)
                nc.vector.tensor_tensor(bsp[:], bsp[:], sp3[:, 2], ALU.mult)
                nc.vector.tensor_scalar(bsp[:], bsp[:], float(beta_t), None, ALU.mult)
                # c2v = (sign * beta*sprod) * exclmin
                bb = bsp[:, None, :, :].to_broadcast([PB, DC, CHUNK_BLKS, BL])
                nc.vector.tensor_tensor(sg[:], sg[:], bb, ALU.mult)
                nc.vector.tensor_tensor(cs, sg[:], ex[:], ALU.mult)

            for t in range(T):
                beta_t = float(beta[t])
                alpha_t = float(alpha[t])
                alpha_p = float(alpha[t - 1]) if t > 0 else 0.0
                cdt, cdvt = cdrs[t % 2], cdrv[t % 2]
                udt, udvt = udrs[t % 2], udrv[t % 2]

                # --- check phase; c2v slots 2..5 -> DRAM by halves ---
                for ck in range(NCHUNK):
                    check_chunk(t, ck, beta_t, alpha_p)
                    if ck == NCHUNK // 2 - 1:
                        for j in range(2, DC):
                            nc.sync.dma_start(
                                cdvt[j - 2][:, :16, :], C[:, j, :16, :]
                            )
                for j in range(2, DC):
                    nc.sync.dma_start(cdvt[j - 2][:, 16:, :], C[:, j, 16:, :])

                last = t == T - 1
                for h in range(4):
                    hs = slice(h * 8, (h + 1) * 8)
                    ls = slice(h * 64, (h + 1) * 64)
                    gm = gbp.tile([PB, 2, 8, BL], F32, tag="gm", name="gm")
                    gh = gbp.tile([PB, 2, 8, BL], F32, tag="gh", name="gh")
                    for pi in range(2):
                        nc.gpsimd.dma_gather(
                            gm[:, pi], cdt, ix1[pi][:, ls], M // 4, M // 4, BL,
                            single_packet=False, queue_num=qn(),
                        )
                        nc.gpsimd.dma_gather(
                            gh[:, pi], cdt, ix2[pi][:, ls], M // 4, M // 4, BL,
                            single_packet=False, queue_num=qn(),
                        )
                    up = U[:, 0:2, hs, :]
                    nc.vector.tensor_tensor(up, C[:, 0:2, hs, :], gm[:], ALU.add)
                    nc.vector.tensor_tensor(up, up, gh[:], ALU.add)
                    if not last:
                        # u = llr + alpha * s
                        nc.vector.scalar_tensor_tensor(
                            up, up, alpha_t, LV[:, :, hs, :], ALU.mult, ALU.add
                        )
                        nc.sync.dma_start(udvt[:, :, hs, :], up)
                    else:
                        # posterior = llr + s ; bits = posterior < 0
                        nc.vector.tensor_tensor(up, up, LV[:, :, hs, :], ALU.add)
                        bt = bip.tile([PB, 2, 8, BL], I32, tag="bt", name="bt")
                        nc.vector.tensor_scalar(bt[:], up, 0.0, None, ALU.is_lt)
                        for pi in range(2):
                            nc.sync.dma_start(post_d[pi][:, hs, :], U[:, pi, hs, :])
                        nc.sync.dma_start(bitv[:, :, hs, :], bt[:])

                if not last:
                    # --- crossing 2: u -> position order, slots 2..5 ---
                    for h in range(4):
                        for i in range(4):
                            nc.gpsimd.dma_gather(
                                U[:, 2 + i, h * 8 : (h + 1) * 8, :],
                                udt,
                                ixu[i][:, h * 64 : (h + 1) * 64],
                                M // 4, M // 4, BL,
                                single_packet=False, queue_num=qn(),
                            )

    nc.compile()
    return nc


def _prepare(llr, edge_v, edge_c, beta, alpha):
    ix1, ix2, ixu, vid_of_fr = _derive_graph(edge_v, edge_c)
    ix1w = np.stack([_wrap_idx(ix1[i]) for i in range(2)])
    ix2w = np.stack([_wrap_idx(ix2[i]) for i in range(2)])
    ixuw = np.stack([_wrap_idx(ixu[i]) for i in range(4)])

    llr = np.asarray(llr, dtype=np.float32)
    in_maps = []
    for k in range(NCORES):
        llr_t = np.ascontiguousarray(llr[k * BL : (k + 1) * BL, vid_of_fr].T)
        in_maps.append({"llr_t": llr_t, "ix1": ix1w, "ix2": ix2w, "ixu": ixuw})
    return in_maps, vid_of_fr


def _assemble(results, vid_of_fr):
    posterior = np.empty((B, N), dtype=np.float32)
    bits = np.empty((B, N), dtype=np.int32)
    for k in range(NCORES):
        pd = results[k]["post"].reshape(N, BL)  # row = pi*4096 + p*32 + g
        bd = results[k]["bits"].reshape(N, BL)
        posterior[k * BL : (k + 1) * BL, vid_of_fr] = pd.T
        bits[k * BL : (k + 1) * BL, vid_of_fr] = bd.T
    return bits, posterior


def _run(llr, edge_v, edge_c, beta, alpha, trace=False, tmpdir=None):
    in_maps, vid_of_fr = _prepare(llr, edge_v, edge_c, beta, alpha)
    nc = _build_program(np.asarray(alpha, np.float32), np.asarray(beta, np.float32))
    res = run_bass_kernel_spmd(
        nc, in_maps, list(range(NCORES)), trace=trace, tmpdir=tmpdir
    )
    return _assemble(res.results, vid_of_fr), res


def kernel(llr, edge_v, edge_c, beta, alpha):
    (bits, posterior), _ = _run(llr, edge_v, edge_c, beta, alpha, trace=False)
    return bits, posterior



# revision 5
# speedup vs baseline: 1.0975x; 1.0975x over previous
"""Trainium2 Bass kernel for the neural 2D min-sum LDPC decoder problem.

Strategy (v3)
-------------
Data-parallel over the batch: B=512 codewords, 64 per NeuronCore (8 cores).
Per core, per-edge state lives in SBUF with the graph on the partition axis
and the 64-batch on the free axis (256B rows).

The Tanner graph (edge_v/edge_c) is 6-regular on checks, 3-regular on
variables, built from 3 "layers": sorting each check's edges by edge id
puts exactly one edge of every variable in slots {0,1}, {2,3}, {4,5}.
Variables are relabeled by their slot-{0,1} position, which makes the
layer-0 part of both crossings contiguous.

v3 restructure vs v2: the check phase no longer computes x = u - alpha*c2v.
Instead the v2c messages are formed in the gather window using the
self-cancellation x_e = llr + alpha*(sum of the OTHER two edges' c2v):
  window (after check):  gm/gh gathers bring c2v of layers 1/2 into
    variable order; x_l0 = llr+a*(gm+gh) feeds abs/sign/pair-min tiles
    consumed by the next check phase; x_l1 = llr+a*(C0+gh) and
    x_l2 = llr+a*(C0+gm) go to DRAM.
  crossing 2: 4-block dst groups gather x into U slots 2..5, overlapping
    the next check phase chunk-by-chunk (Tile subtile deps).
All DVE work for x/pair-0 is thereby moved off the check-phase critical
path into the Pool-bound gather windows.
"""

import sys

for _p in ("/opt/trn_rl_repo",):
    if _p not in sys.path:
        sys.path.insert(0, _p)

import numpy as np

import concourse.bass as bass
import concourse.bacc as bacc
import concourse.mybir as mybir
import concourse.tile as tile
from concourse.bass_utils import run_bass_kernel_spmd

N = 8192          # variable nodes
M = 4096          # check nodes
DC = 6            # check degree (slots)
DV = 3            # variable degree
E = N * DV
B = 512
T = 10
NCORES = 8
BL = B // NCORES  # 64
PB = 128
GB_ = M // PB     # 32 blocks per slot array
CHUNK_BLKS = 4
NCHUNK = GB_ // CHUNK_BLKS

F32 = mybir.dt.float32
I32 = mybir.dt.int32
I16 = mybir.dt.int16
ALU = mybir.AluOpType
ACTF = mybir.ActivationFunctionType


def _derive_graph(edge_v: np.ndarray, edge_c: np.ndarray):
    """Host-side index derivation (layered 6-regular/3-regular graph)."""
    edge_v = np.asarray(edge_v, dtype=np.int64)
    edge_c = np.asarray(edge_c, dtype=np.int64)
    assert edge_v.shape == (E,) and edge_c.shape == (E,)

    order = np.argsort(edge_c, kind="stable")
    assert (edge_c[order] == np.repeat(np.arange(M), DC)).all(), (
        "graph is not 6-regular on checks"
    )
    slot_edge = order.reshape(M, DC).T.copy()  # [DC, M] edge id at (slot j, check c)

    # per-edge position
    j_of_e = np.empty(E, dtype=np.int64)
    c_of_e = np.empty(E, dtype=np.int64)
    for j in range(DC):
        j_of_e[slot_edge[j]] = j
        c_of_e[slot_edge[j]] = np.arange(M)

    # each variable must have exactly one edge in slots {0,1}, {2,3}, {4,5}
    layer_of_e = j_of_e // 2
    ve = np.full((N, 3), -1, dtype=np.int64)
    for lay in range(3):
        sel = np.where(layer_of_e == lay)[0]
        vs = edge_v[sel]
        assert len(np.unique(vs)) == N, f"layer {lay} is not a permutation"
        ve[vs, lay] = sel
    assert (ve >= 0).all()

    # storage row helpers (p-major: row = (c%128)*32 + c//128)
    rowmaj = (c_of_e % PB) * GB_ + (c_of_e // PB)
    # c2v DRAM buffer holds slots 2..5 only
    cdrow = (j_of_e - 2) * M + rowmaj          # valid for slots 2..5
    # u/llr DRAM row of a variable = its slot-{0,1} position
    fr_of_e = j_of_e * M + rowmaj              # valid for slots 0..1
    fr_of_v = fr_of_e[ve[:, 0]]                # [N]

    # x-build gathers (dst = parity pi, list pos = check c): variable at
    # (j=pi, c) -> cdram rows of its layer-1 / layer-2 edges
    ix1 = np.empty((2, M), dtype=np.int16)
    ix2 = np.empty((2, M), dtype=np.int16)
    # crossing-2 gathers (dst slot j=2..5, list pos = c): x DRAM row of v(j,c)
    ixu = np.empty((4, M), dtype=np.int16)
    for pi in range(2):
        e = slot_edge[pi]                      # layer-0 edge at (pi, c)
        v = edge_v[e]
        ix1[pi] = cdrow[ve[v, 1]]
        ix2[pi] = cdrow[ve[v, 2]]
    for j in range(2, DC):
        v = edge_v[slot_edge[j]]
        ixu[j - 2] = fr_of_v[v]

    # host llr/output mapping: variable id at each u/llr DRAM row
    vid_of_fr = np.empty(N, dtype=np.int64)
    vid_of_fr[fr_of_v] = np.arange(N)
    return ix1, ix2, ixu, vid_of_fr


def _wrap_idx(idx_m: np.ndarray) -> np.ndarray:
    """dma_gather index layout: list position k at [k%16, k//16],
    replicated across the 8 groups of 16 partitions."""
    w = idx_m.reshape(M // 16, 16).T
    return np.tile(w, (PB // 16, 1)).copy()


def _build_program(alpha: np.ndarray, beta: np.ndarray) -> bacc.Bacc:
    nc = bacc.Bacc(num_swdge_queues=4)

    llr_t = nc.dram_tensor("llr_t", [N, BL], F32, kind="ExternalInput").ap()
    ix1_d = nc.dram_tensor("ix1", [2, PB, M // 16], I16, kind="ExternalInput").ap()
    ix2_d = nc.dram_tensor("ix2", [2, PB, M // 16], I16, kind="ExternalInput").ap()
    ixu_d = nc.dram_tensor("ixu", [4, PB, M // 16], I16, kind="ExternalInput").ap()
    post_d = nc.dram_tensor("post", [2, PB, GB_, BL], F32, kind="ExternalOutput").ap()
    bits_d = nc.dram_tensor("bits", [2, PB, GB_, BL], I32, kind="ExternalOutput").ap()
    # c2v slots 2..5, ping-pong; x1/x2 (v2c messages of layers 1/2 in
    # variable order), ping-pong
    cdrs = [
        nc.dram_tensor("cda", [4 * M, BL], F32).ap(),
        nc.dram_tensor("cdb", [4 * M, BL], F32).ap(),
    ]
    x1rs = [
        nc.dram_tensor("x1a", [N, BL], F32).ap(),
        nc.dram_tensor("x1b", [N, BL], F32).ap(),
    ]
    x2rs = [
        nc.dram_tensor("x2a", [N, BL], F32).ap(),
        nc.dram_tensor("x2b", [N, BL], F32).ap(),
    ]
    cdrv = [c.rearrange("(j p g) e -> j p g e", j=4, p=PB) for c in cdrs]
    x1rv = [u.rearrange("(pi p g) e -> p pi g e", pi=2, p=PB) for u in x1rs]
    x2rv = [u.rearrange("(pi p g) e -> p pi g e", pi=2, p=PB) for u in x2rs]
    bitv = bits_d.rearrange("pi p g e -> p pi g e")

    QN = [0]

    def qn():
        # one queue per DMA-sem lane-pair: Tile locks each SWDGE sem lane to
        # a single queue, and lanes are assigned round-robin per gather.
        q = (QN[0] % 8) // 2
        QN[0] += 1
        return q

    S1 = CHUNK_BLKS * BL  # free elems per slot per chunk (256)

    with tile.TileContext(nc) as tc:
        with (
            tc.tile_pool(name="persist", bufs=1) as pp,
            tc.tile_pool(name="gbp", bufs=2) as gbp,
            tc.tile_pool(name="xdp", bufs=2) as xdp,
            tc.tile_pool(name="wtp", bufs=1) as wtp,
            tc.tile_pool(name="tmp", bufs=1) as tp,
            tc.tile_pool(name="ps", bufs=1, space="PSUM") as psp,
        ):
            ix1 = [pp.tile([PB, M // 16], I16, tag=f"ix1{i}", name=f"ix1{i}") for i in range(2)]
            ix2 = [pp.tile([PB, M // 16], I16, tag=f"ix2{i}", name=f"ix2{i}") for i in range(2)]
            ixu = [pp.tile([PB, M // 16], I16, tag=f"ixu{i}", name=f"ixu{i}") for i in range(4)]
            for i in range(2):
                nc.sync.dma_start(ix1[i][:], ix1_d[i])
                nc.sync.dma_start(ix2[i][:], ix2_d[i])
            for i in range(4):
                nc.sync.dma_start(ixu[i][:], ixu_d[i])

            # llr in variable(-row) order, parity-split: [128, 2, 32, 64]
            LV = pp.tile([PB, 2, GB_, BL], F32, tag="lv", name="lv")
            nc.sync.dma_start(
                LV[:], llr_t.rearrange("(pi p g) e -> p pi g e", pi=2, p=PB)
            )
            # x at positions, slots 2..5 (layers 1/2), gathered per iteration
            U = pp.tile([PB, 4, GB_, BL], F32, tag="u", name="u")
            # c2v (all 6 slots, check order)
            C = pp.tile([PB, DC, GB_, BL], F32, tag="c", name="c")
            # pair-0 (slots 0/1) magnitudes, signs, pair-min, pair-signprod
            M01 = pp.tile([PB, 2, GB_, BL], F32, tag="m01", name="m01")
            S01 = pp.tile([PB, 2, GB_, BL], F32, tag="s01", name="s01")
            PP0 = pp.tile([PB, GB_, BL], F32, tag="pp0", name="pp0")
            SP0 = pp.tile([PB, GB_, BL], F32, tag="sp0", name="sp0")

            # t=0: x(0) = llr at every edge.
            # pair-0 tiles from LV; slots 2..5 gathered from llr_t.
            nc.scalar.activation(M01[:], LV[:], ACTF.Abs)
            nc.scalar.activation(S01[:], LV[:], ACTF.Sign)
            nc.vector.tensor_tensor(PP0[:], M01[:, 0], M01[:, 1], ALU.min)
            nc.vector.tensor_tensor(SP0[:], S01[:, 0], S01[:, 1], ALU.mult)
            for h in range(2):
                for i in range(4):
                    nc.gpsimd.dma_gather(
                        U[:, i, h * 16 : (h + 1) * 16, :],
                        llr_t,
                        ixu[i][:, h * 128 : (h + 1) * 128],
                        M // 2, M // 2, BL,
                        single_packet=False, queue_num=qn(),
                    )

            def check_chunk(ck, beta_t, cdvt):
                """min-sum check update for chunk ck (CHUNK_BLKS blocks).
                Consumes U (x at slots 2..5), M01/S01/PP0/SP0 (pair 0);
                writes C (c2v, all 6 slots) and DMAs slots 2..5 to DRAM."""
                b0 = ck * CHUNK_BLKS
                bs = slice(b0, b0 + CHUNK_BLKS)
                us = U[:, :, bs, :]
                mgc = tp.tile([PB, 4, CHUNK_BLKS, BL], F32, tag="mgc", name="mgc")
                sgc = tp.tile([PB, 4, CHUNK_BLKS, BL], F32, tag="sgc", name="sgc")
                nc.scalar.activation(mgc[:], us, ACTF.Abs)
                nc.scalar.activation(sgc[:], us, ACTF.Sign)
                pp12 = tp.tile([PB, 2, CHUNK_BLKS, BL], F32, tag="pp12", name="pp12")
                sp12 = tp.tile([PB, 2, CHUNK_BLKS, BL], F32, tag="sp12", name="sp12")
                nc.vector.tensor_tensor(pp12[:], mgc[:, 0::2], mgc[:, 1::2], ALU.min)
                nc.vector.tensor_tensor(sp12[:], sgc[:, 0::2], sgc[:, 1::2], ALU.mult)
                # leave-one-pair-out mins: qq[0]=min(pp12[0],pp12[1]);
                # qq[1]=min(PP0,pp12[1]); qq[2]=min(PP0,pp12[0])
                qq = tp.tile([PB, 3, CHUNK_BLKS, BL], F32, tag="qq", name="qq")
                nc.vector.tensor_tensor(qq[:, 0], pp12[:, 0], pp12[:, 1], ALU.min)
                pv = pp12[:]
                pswap = bass.AP(
                    pv.tensor, pv.offset + S1,
                    [pv.ap[0], [-S1, 2], [1, S1]],
                )
                p0b = (PP0[:, bs, :].rearrange("p b e -> p (b e)")[:, None, :]
                       .to_broadcast([PB, 2, S1]))
                nc.vector.tensor_tensor(
                    qq[:, 1:3].rearrange("p a b e -> p a (b e)"), pswap, p0b, ALU.min
                )
                # total sign product * beta
                bsp = tp.tile([PB, CHUNK_BLKS, BL], F32, tag="bsp", name="bsp")
                nc.vector.tensor_tensor(bsp[:], SP0[:, bs, :], sp12[:, 0], ALU.mult)
                nc.vector.scalar_tensor_tensor(
                    bsp[:], bsp[:], float(beta_t), sp12[:, 1], ALU.mult, ALU.mult
                )
                bb01 = bsp[:, None, :, :].to_broadcast([PB, 2, CHUNK_BLKS, BL])
                bb45 = bsp[:, None, :, :].to_broadcast([PB, 4, CHUNK_BLKS, BL])
                # leave-one-out mins
                ex01 = psp.tile([PB, 2, CHUNK_BLKS, BL], F32, tag="ex01", name="ex01")
                ex45 = psp.tile([PB, 4, CHUNK_BLKS, BL], F32, tag="ex45", name="ex45")
                mt = M01[:].tensor
                m01sw = bass.AP(
                    mt, M01[:].offset + GB_ * BL + b0 * BL,
                    [M01[:].ap[0], [-GB_ * BL, 2], [BL, CHUNK_BLKS], [1, BL]],
                )
                q0b = (qq[:, 0].rearrange("p b e -> p (b e)")[:, None, :]
                       .to_broadcast([PB, 2, S1]))
                nc.vector.tensor_tensor(
                    ex01[:].rearrange("p a b e -> p a (b e)"),
                    m01sw.rearrange("p a b e -> p a (b e)"), q0b, ALU.min
                )
                mv = mgc[:]
                msw = bass.AP(
                    mv.tensor, mv.offset + S1,
                    [mv.ap[0], [2 * S1, 2], [-S1, 2], [1, S1]],
                )
                qb = (qq[:, 1:3].rearrange("p a b e -> p a (b e)")[:, :, None, :]
                      .to_broadcast([PB, 2, 2, S1]))
                nc.vector.tensor_tensor(
                    ex45[:].rearrange("p (a b) c e -> p a b (c e)", a=2), msw, qb, ALU.min
                )
                # c2v = (sign * beta*sprod) * exclmin
                sgb01 = tp.tile([PB, 2, CHUNK_BLKS, BL], F32, tag="sgb01", name="sgb01")
                nc.vector.tensor_tensor(sgb01[:], S01[:, :, bs, :], bb01, ALU.mult)
                nc.vector.tensor_tensor(C[:, 0:2, bs, :], sgb01[:], ex01[:], ALU.mult)
                nc.vector.tensor_tensor(sgc[:], sgc[:], bb45, ALU.mult)
                nc.vector.tensor_tensor(C[:, 2:6, bs, :], sgc[:], ex45[:], ALU.mult)
                for j in range(2, DC):
                    nc.sync.dma_start(cdvt[j - 2][:, bs, :], C[:, j, bs, :])

            for t in range(T):
                beta_t = float(beta[t])
                alpha_t = float(alpha[t])
                cdt, cdvt = cdrs[t % 2], cdrv[t % 2]
                x1t, x1vt = x1rs[t % 2], x1rv[t % 2]
                x2t, x2vt = x2rs[t % 2], x2rv[t % 2]
                last = t == T - 1

                # --- check phase ---
                for ck in range(NCHUNK):
                    check_chunk(ck, beta_t, cdvt)

                # --- window: gm/gh gathers + x build (or posterior) ---
                for h in range(4):
                    hs = slice(h * 8, (h + 1) * 8)
                    ls = slice(h * 64, (h + 1) * 64)
                    gm = gbp.tile([PB, 2, 8, BL], F32, tag="gm", name="gm")
                    gh = gbp.tile([PB, 2, 8, BL], F32, tag="gh", name="gh")
                    for pi in range(2):
                        nc.gpsimd.dma_gather(
                            gm[:, pi], cdt, ix1[pi][:, ls], M // 4, M // 4, BL,
                            single_packet=False, queue_num=qn(),
                        )
                        nc.gpsimd.dma_gather(
                            gh[:, pi], cdt, ix2[pi][:, ls], M // 4, M // 4, BL,
                            single_packet=False, queue_num=qn(),
                        )
                    lvh = LV[:, :, hs, :]
                    c0h = C[:, 0:2, hs, :]
                    if last:
                        # posterior = llr + (C0 + gm + gh); bits = post < 0
                        w1 = wtp.tile([PB, 2, 8, BL], F32, tag="w1", name="w1")
                        nc.vector.tensor_tensor(w1[:], gm[:], gh[:], ALU.add)
                        nc.vector.tensor_tensor(w1[:], w1[:], c0h, ALU.add)
                        nc.vector.tensor_tensor(w1[:], w1[:], lvh, ALU.add)
                        bt = xdp.tile([PB, 2, 8, BL], I32, tag="xd1", name="bt")
                        nc.vector.tensor_scalar(bt[:], w1[:], 0.0, None, ALU.is_lt)
                        for pi in range(2):
                            nc.sync.dma_start(post_d[pi][:, hs, :], w1[:, pi])
                        nc.sync.dma_start(bitv[:, :, hs, :], bt[:])
                    else:
                        # x_l0 = llr + a*(gm+gh) -> pair-0 tiles
                        # x_l1 = llr + a*(C0+gh) -> x1 DRAM
                        # x_l2 = llr + a*(C0+gm) -> x2 DRAM
                        w1 = wtp.tile([PB, 2, 8, BL], F32, tag="w1", name="w1")
                        w2 = wtp.tile([PB, 2, 8, BL], F32, tag="w2", name="w2")
                        nc.vector.tensor_tensor(w1[:], gm[:], gh[:], ALU.add)
                        nc.vector.scalar_tensor_tensor(
                            w2[:], w1[:], alpha_t, lvh, ALU.mult, ALU.add
                        )
                        nc.scalar.activation(M01[:, :, hs, :], w2[:], ACTF.Abs)
                        nc.scalar.activation(S01[:, :, hs, :], w2[:], ACTF.Sign)
                        nc.vector.tensor_tensor(
                            PP0[:, hs, :], M01[:, 0, hs, :], M01[:, 1, hs, :], ALU.min
                        )
                        nc.vector.tensor_tensor(
                            SP0[:, hs, :], S01[:, 0, hs, :], S01[:, 1, hs, :], ALU.mult
                        )
                        xd1 = xdp.tile([PB, 2, 8, BL], F32, tag="xd1", name="xd1")
                        xd2 = xdp.tile([PB, 2, 8, BL], F32, tag="xd2", name="xd2")
                        nc.vector.tensor_tensor(w1[:], c0h, gh[:], ALU.add)
                        nc.vector.scalar_tensor_tensor(
                            xd1[:], w1[:], alpha_t, lvh, ALU.mult, ALU.add
                        )
                        nc.sync.dma_start(x1vt[:, :, hs, :], xd1[:])
                        nc.vector.tensor_tensor(w1[:], c0h, gm[:], ALU.add)
                        nc.vector.scalar_tensor_tensor(
                            xd2[:], w1[:], alpha_t, lvh, ALU.mult, ALU.add
                        )
                        nc.sync.dma_start(x2vt[:, :, hs, :], xd2[:])

                if not last:
                    # --- crossing 2: x -> position order, slots 2..5,
                    # 4-block dst groups so the next check phase overlaps ---
                    for g in range(8):
                        gs = slice(g * 32, (g + 1) * 32)
                        ds = slice(g * 4, (g + 1) * 4)
                        for i in range(4):
                            nc.gpsimd.dma_gather(
                                U[:, i, ds, :],
                                x1t if i < 2 else x2t,
                                ixu[i][:, gs],
                                M // 8, M // 8, BL,
                                single_packet=False, queue_num=qn(),
                            )

    nc.compile()
    return nc


def _prepare(llr, edge_v, edge_c, beta, alpha):
    ix1, ix2, ixu, vid_of_fr = _derive_graph(edge_v, edge_c)
    ix1w = np.stack([_wrap_idx(ix1[i]) for i in range(2)])
    ix2w = np.stack([_wrap_idx(ix2[i]) for i in range(2)])
    ixuw = np.stack([_wrap_idx(ixu[i]) for i in range(4)])

    llr = np.asarray(llr, dtype=np.float32)
    in_maps = []
    for k in range(NCORES):
        llr_t = np.ascontiguousarray(llr[k * BL : (k + 1) * BL, vid_of_fr].T)
        in_maps.append({"llr_t": llr_t, "ix1": ix1w, "ix2": ix2w, "ixu": ixuw})
    return in_maps, vid_of_fr


def _assemble(results, vid_of_fr):
    posterior = np.empty((B, N), dtype=np.float32)
    bits = np.empty((B, N), dtype=np.int32)
    for k in range(NCORES):
        pd = results[k]["post"].reshape(N, BL)  # row = pi*4096 + p*32 + g
        bd = results[k]["bits"].reshape(N, BL)
        posterior[k * BL : (k + 1) * BL, vid_of_fr] = pd.T
        bits[k * BL : (k + 1) * BL, vid_of_fr] = bd.T
    return bits, posterior


def _run(llr, edge_v, edge_c, beta, alpha, trace=False, tmpdir=None):
    in_maps, vid_of_fr = _prepare(llr, edge_v, edge_c, beta, alpha)
    nc = _build_program(np.asarray(alpha, np.float32), np.asarray(beta, np.float32))
    res = run_bass_kernel_spmd(
        nc, in_maps, list(range(NCORES)), trace=trace, tmpdir=tmpdir
    )
    return _assemble(res.results, vid_of_fr), res


def kernel(llr, edge_v, edge_c, beta, alpha):
    (bits, posterior), _ = _run(llr, edge_v, edge_c, beta, alpha, trace=False)
    return bits, posterior
